# revision 33
# baseline (speedup 1.0000x reference)
"""CapsuleLayer dynamic-routing kernel for 8 Trainium2 NeuronCores.

I-sharding: each core owns IL=144 of the 1152 input capsules.

Layouts (b on SBUF partitions everywhere):
  hat[b, (d, i, n)] bf16   73728 elem/part, SBUF-resident
  e/ee/c[b, (i, n)] bf16   routing logits / exp / weights
  s/U/out[b, (d, n)]       capsule vectors (d-major so d-reduces are contiguous)

Create: s0 = dense GEMM over (i,k) (9 K=128 matmuls); hat_i via K=8 matmuls
with compact streamed W at base partition 0; PSUM evictions split DVE/ACT.
Routing: tree-reduces in bf16 2x mode, big passes split DVE (11/18) and
GPSIMD/Pool (7/18); AllReduce via collective cores with bf16 payload.
"""

import os
import numpy as np
import ml_dtypes

import concourse.bass as bass
import concourse.bacc as bacc
import concourse.tile as tile
import concourse.mybir as mybir
from concourse import bass_utils

B, I, DIN = 128, 1152, 8
N, D = 32, 16
ND = N * D  # 512
NCORES = 8
IL = I // NCORES  # 144
EPS = 1e-7
ROUTINGS = 3
F32 = mybir.dt.float32
BF16 = mybir.dt.bfloat16

NCH = 9            # dense GEMM K-chunks (IL*DIN/128)
TRI = 2            # i's per PSUM eviction group (2 banks)
NTRI = IL // TRI   # 72
IC = 8             # i-chunk for routing passes
NICH = IL // IC    # 18
def _mk_chunks(pattern):
    out = []
    i0 = 0
    for c in pattern:
        ic = 16 if c == "D" else 8
        out.append((i0, ic, c == "P"))
        i0 += ic
    assert i0 == IL
    return out


# 18 chunks of 8; booleans mark Pool-owned chunks
_POOL_A = frozenset((2, 5, 7, 10, 12, 14, 16))
_POOL_B = frozenset((1, 3, 5, 8, 10, 12, 14, 16))
CHUNKS_A = [(8 * t, 8, t in _POOL_A) for t in range(18)]
CHUNKS_B = [(8 * t, 8, t in _POOL_B) for t in range(18)]


def _ap(ap: bass.AP, dims) -> bass.AP:
    """Rebuild `ap` with explicit free [step,count] dims (partition dim kept)."""
    return bass.AP(tensor=ap.tensor, offset=ap.offset, ap=[ap.ap[0]] + list(dims))


def _off(ap: bass.AP, off, dims) -> bass.AP:
    """Like _ap but with an extra element offset into the free space."""
    return bass.AP(tensor=ap.tensor, offset=ap.offset + off,
                   ap=[ap.ap[0]] + list(dims))


def build_nc():
    nc = bacc.Bacc(
        "TRN2",
        target_bir_lowering=False,
        debug=False,
        enable_asserts=True,
        num_devices=NCORES,
    )
    xd_d = nc.dram_tensor("xd", [128, NCH, B], BF16, kind="ExternalInput").ap()
    wf_d = nc.dram_tensor("wf", [128, NCH, ND], BF16, kind="ExternalInput").ap()
    xz_d = nc.dram_tensor("xz", [128, IL, B], BF16, kind="ExternalInput").ap()
    out_d = nc.dram_tensor("out", [B // NCORES, ND], F32, kind="ExternalOutput").ap()

    with tile.TileContext(nc) as tc:
        with (
            tc.tile_pool(name="big", bufs=1) as big,
            tc.tile_pool(name="ps", bufs=3, space="PSUM") as pspool,
            tc.tile_pool(name="ps0", bufs=1, space="PSUM") as ps0pool,
            tc.tile_pool(name="dram", bufs=1, space="DRAM") as dram,
        ):
            lp = nc.allow_low_precision(reason="bf16 routing pipeline")
            lp.__enter__()

            hat = big.tile([B, D, IL, N], BF16)        # 144 KB/part
            s_send = big.tile([B, ND], BF16)
            s_sb = big.tile([B, ND], BF16)
            u_bf = big.tile([B, ND], BF16)
            s2f = big.tile([B, ND], BF16)
            sacc_d = big.tile([B, ND], BF16)
            sacc_p = big.tile([B, ND], BF16)
            outv = big.tile([B, ND], F32)              # 2 KB
            nsum = big.tile([B, IL], F32)
            rcp_t = big.tile([B, IL], BF16)
            sq = big.tile([B, 5, N], F32)              # squash scratch
            sq_s2, sq_a, sq_r, sq_t = (sq[:, j, :] for j in range(4))
            eps_t = sq[:, 4, 0:1]
            scale_bf = big.tile([B, N], BF16)

            nc.vector.memset(eps_t, EPS)

            # ---------------- create ----------------
            xd, xd_free = tc.tile([B, NCH, B], BF16, name="xd_t")
            wf, wf_free = tc.tile([B, NCH, ND], BF16, name="wf_t")
            xz, xz_free = tc.tile([B, IL, B], BF16, name="xz_t")
            nc.sync.dma_start(out=xd[:], in_=xd_d[:])
            nc.sync.dma_start(out=wf[:], in_=wf_d[:])
            for h in range(6):
                nc.sync.dma_start(
                    out=xz[:, 24 * h:24 * (h + 1), :],
                    in_=xz_d[:, 24 * h:24 * (h + 1), :])

            s0ps = ps0pool.tile([B, ND], F32)
            for c in range(NCH):
                nc.tensor.matmul(
                    s0ps[:], lhsT=xd[:, c, :], rhs=wf[:, c, :],
                    start=(c == 0), stop=(c == NCH - 1),
                )
            nc.scalar.mul(out=s_send[:], in_=s0ps[:], mul=1.0 / N)

            def allreduce_s(idx, scatter=False):
                ar_in = dram.tile([B, ND], BF16, tag=f"arin{idx}",
                                  name=f"arin{idx}")
                osz = B // NCORES if scatter else B
                ar_out = dram.tile([osz, ND], BF16, tag=f"arout{idx}",
                                   name=f"arout{idx}")
                nc.gpsimd.dma_start(out=ar_in[:], in_=s_send[:])
                nc.gpsimd.collective_compute(
                    "ReduceScatter" if scatter else "AllReduce",
                    mybir.AluOpType.add,
                    replica_groups=[list(range(NCORES))],
                    ins=[ar_in.opt()],
                    outs=[ar_out.opt()],
                )
                nc.gpsimd.dma_start(out=s_sb[0:osz, :], in_=ar_out[:])

            allreduce_s(0)

            for t in range(NTRI):
                i0 = t * TRI
                ps = pspool.tile([B, TRI, ND], F32, tag="ps", name="ps")
                for j in range(TRI):
                    i = i0 + j
                    nc.tensor.matmul(
                        ps[:, j, :],
                        lhsT=xz[:, i, :],
                        rhs=wf[:, i // 16, :],
                        start=True, stop=True,
                    )
                # evict psum[(i,d,n)] -> hat[b, (d, i0:i0+TRI, n)]
                src = _ap(ps[:], [[N, D], [ND, TRI], [1, N]])
                dst = _ap(hat[:, :, i0:i0 + TRI, :],
                          [[IL * N, D], [N, TRI], [1, N]])
                if t % 2 == 0:
                    nc.scalar.copy(out=dst, in_=src)
                else:
                    nc.vector.tensor_copy(dst, src)

            xz_free()
            wf_free()
            xd_free()

            e_t, e_free = tc.tile([B, IL, N], BF16, name="e_t")
            ee_t, ee_free = tc.tile([B, IL, N], BF16, name="ee_t")
            scr_d, sd_free = tc.tile([B, D, IC, N], BF16, name="scr_d")
            scr_p, sp_free = tc.tile([B, D, IC, N], BF16, name="scr_p")
            c_t = ee_t

            # ---------------- routing helpers ----------------
            def squash(r, np_=B):
                """s_sb bf16 [b,(d,n)] -> outv f32; update U."""
                nc.vector.tensor_mul(s2f[0:np_, :], s_sb[0:np_, :], s_sb[0:np_, :])
                a = s2f[0:np_, :]
                for w in (256, 128, 64):
                    nc.vector.tensor_add(
                        _ap(a, [[1, w]]), _ap(a, [[1, w]]),
                        _off(a, w, [[1, w]]))
                nc.vector.tensor_add(
                    sq_s2[0:np_, :], _ap(a, [[1, 32]]), _off(a, 32, [[1, 32]]))
                nc.vector.tensor_scalar_add(sq_a[0:np_, :], sq_s2[0:np_, :], 1.0)
                nc.vector.reciprocal(out=sq_r[0:np_, :], in_=sq_a[0:np_, :])
                nc.vector.tensor_mul(sq_r[0:np_, :], sq_r[0:np_, :], sq_s2[0:np_, :])
                nc.scalar.activation(
                    out=sq_t[0:np_, :], in_=sq_s2[0:np_, :],
                    func=mybir.ActivationFunctionType.Sqrt,
                    bias=eps_t[0:np_, :] if np_ != B else eps_t,
                    scale=1.0)
                nc.vector.reciprocal(out=sq_t[0:np_, :], in_=sq_t[0:np_, :])
                nc.vector.tensor_mul(sq_r[0:np_, :], sq_r[0:np_, :], sq_t[0:np_, :])
                nc.vector.tensor_copy(scale_bf[0:np_, :], sq_r[0:np_, :])
                nc.vector.tensor_mul(
                    _ap(outv[0:np_, :], [[N, D], [1, N]]),
                    _ap(s_sb[0:np_, :], [[N, D], [1, N]]),
                    _ap(scale_bf[0:np_, :], [[0, D], [1, N]]))
                if r == 0:
                    nc.vector.tensor_copy(u_bf[:], outv[:])
                elif r < ROUTINGS - 1:
                    nc.vector.tensor_add(u_bf[:], u_bf[:], outv[:])

            def pass_a_chunks(chunks):
                """e[b,(i,n)] = sum_d hat[b,(d,i,n)] * U[b,(d,n)]."""
                for (i0, ic, pool) in chunks:
                    v = nc.gpsimd if pool else nc.vector
                    scr = (scr_p if pool else scr_d)[:]
                    v.tensor_mul(
                        _ap(scr, [[1, D * ic * N]]),
                        _ap(hat[:, :, i0:i0 + ic, :],
                            [[IL * N, D], [N, ic], [1, N]]),
                        _ap(u_bf[:], [[N, D], [0, ic], [1, N]]))
                    for lw in (8, 4, 2):
                        w = lw * ic * N
                        v.tensor_add(
                            _ap(scr, [[1, w]]), _ap(scr, [[1, w]]),
                            _off(scr, w, [[1, w]]))
                    v.tensor_add(
                        _ap(e_t[:, i0:i0 + ic, :], [[1, ic * N]]),
                        _ap(scr, [[1, ic * N]]),
                        _off(scr, ic * N, [[1, ic * N]]))

            def softmax_grp(s0_, s1_, pool):
                H = s1_ - s0_
                nc.scalar.activation(
                    out=ee_t[:, s0_:s1_, :], in_=e_t[:, s0_:s1_, :],
                    func=mybir.ActivationFunctionType.Exp,
                    bias=eps_t, scale=1.0)
                nc.vector.reduce_sum(
                    out=nsum[:, s0_:s1_],
                    in_=_ap(ee_t[:, s0_:s1_, :], [[N, H], [1, N]]),
                    axis=mybir.AxisListType.X)
                nc.vector.reciprocal(out=nsum[:, s0_:s1_],
                                     in_=nsum[:, s0_:s1_])
                nc.vector.tensor_copy(rcp_t[:, s0_:s1_], nsum[:, s0_:s1_])
                vm = nc.gpsimd if pool else nc.vector
                vm.tensor_mul(
                    _ap(c_t[:, s0_:s1_, :], [[1, H * N]]),
                    _ap(ee_t[:, s0_:s1_, :], [[1, H * N]]),
                    _ap(rcp_t[:, s0_:s1_], [[1, H], [0, N]]))

            def pass_b_chunks(chunks):
                """s_partial[b,(d,n)] = sum_i c[b,(i,n)] * hat[b,(d,i,n)]."""
                first = {False: True, True: True}
                for (i0, ic, pool) in chunks:
                    v = nc.gpsimd if pool else nc.vector
                    scr = (scr_p if pool else scr_d)[:]
                    sacc = (sacc_p if pool else sacc_d)[:]
                    v.tensor_mul(
                        _ap(scr, [[1, D * ic * N]]),
                        _ap(hat[:, :, i0:i0 + ic, :],
                            [[IL * N, D], [N, ic], [1, N]]),
                        _ap(c_t[:, i0:i0 + ic, :], [[0, D], [1, ic * N]]))
                    w = ic
                    while w > 1:
                        w //= 2
                        v.tensor_add(
                            _ap(scr, [[ic * N, D], [1, w * N]]),
                            _ap(scr, [[ic * N, D], [1, w * N]]),
                            _off(scr, w * N, [[ic * N, D], [1, w * N]]))
                    if first[pool]:
                        v.tensor_copy(sacc, _ap(scr, [[ic * N, D], [1, N]]))
                        first[pool] = False
                    else:
                        v.tensor_add(
                            sacc, sacc, _ap(scr, [[ic * N, D], [1, N]]))

            # ---------------- routing ----------------
            BL = B // NCORES
            for r in range(ROUTINGS):
                squash(r, BL if r == ROUTINGS - 1 else B)
                if r == ROUTINGS - 1:
                    break
                pass_a_chunks(CHUNKS_A)
                for h in range(2):
                    softmax_grp(h * 72, (h + 1) * 72, pool=(h == 1))
                pass_b_chunks(CHUNKS_B)
                nc.vector.tensor_add(s_send[:], sacc_d[:], sacc_p[:])
                allreduce_s(r + 1, scatter=(r + 1 == ROUTINGS - 1))

            nc.sync.dma_start(out=out_d[:], in_=outv[0:BL, :])
            for f in (sp_free, sd_free, ee_free, e_free):
                f()
            lp.__exit__(None, None, None)

    nc.compile()
    return nc


_NC_CACHE = None


def make_in_maps(inputs: np.ndarray, W: np.ndarray) -> list[dict]:
    x = np.ascontiguousarray(inputs, dtype=np.float32)
    w = np.ascontiguousarray(W, dtype=np.float32)
    bf = ml_dtypes.bfloat16
    in_maps = []
    for c in range(NCORES):
        sl = slice(c * IL, (c + 1) * IL)
        xc = x[:, sl, :]                    # [B, IL, DIN]
        wc = w[:, sl, :, :]                 # [N, IL, D, DIN]
        # xz[p, i, b]: zero-padded x columns; p = 8*(i%16)+k holds x[b,i,k]
        xz = np.zeros((128, IL, B), dtype=np.float32)
        ii = np.arange(IL)
        for k in range(DIN):
            xz[8 * (ii % 16) + k, ii, :] = xc[:, ii, k].T
        xz = xz.astype(bf)
        # xd[p, ch, b]: row r=128*ch+p -> (i=r//8, k=r%8)
        xdr = np.ascontiguousarray(
            xc.transpose(1, 2, 0).reshape(NCH, 128, B)
            .transpose(1, 0, 2)).astype(bf)
        # wf[p, ch, d*32+n]: same row map; cols (d,n)
        wfr = np.ascontiguousarray(
            wc.transpose(1, 3, 2, 0).reshape(IL * DIN, ND)
            .reshape(NCH, 128, ND).transpose(1, 0, 2)).astype(bf)
        in_maps.append({"xd": xdr, "wf": wfr, "xz": xz})
    return in_maps


def finalize_output(outs: list) -> np.ndarray:
    # per-core out[bl, d*32+n] slices -> [B, N, D]
    full = np.concatenate([np.asarray(o, dtype=np.float32) for o in outs], axis=0)
    return full.reshape(B, D, N).transpose(0, 2, 1)


def kernel(inputs: np.ndarray, W: np.ndarray) -> np.ndarray:
    global _NC_CACHE
    if _NC_CACHE is None:
        _NC_CACHE = build_nc()
    nc = _NC_CACHE

    in_maps = make_in_maps(inputs, W)

    trace = bool(int(os.environ.get("CAPS_TRACE", "0")))
    res = bass_utils.run_bass_kernel_spmd(
        nc, in_maps, core_ids=list(range(NCORES)), trace=trace)
    if trace and res.exec_time_ns is not None:
        print(f"HW exec time: {res.exec_time_ns} ns")
    return finalize_output([res.results[c]["out"] for c in range(NCORES)])
